# revision 4
# baseline (speedup 1.0000x reference)
"""Trainium2 Bass kernel for nn_ClusterLoss (topk_masking).

Strategy (8 NeuronCores, data-parallel over the 4096 selected rows):
  - Host shards mc_rows and the corresponding gathered row_scores rows
    across cores (512 rows/core). The gathered rows are negated and the
    column index is packed into the low 14 mantissa bits (value rounded
    to the remaining 9 mantissa bits), so a single VectorE MAX8 pass
    yields both the 3 smallest scores and their column indices.
  - Device, per core: MAX8 per 128-row tile -> top-3 packed values;
    tiny bitwise unpack (indices + quantized values), softmax weights
    via ScalarE Exp, H[idx] gathered with indirect DMA, norm math
    spread across GpSimd/ScalarE/VectorE. Masked-MSE residual and
    squared-norm partials for a 1250-row slice of X/H/C/M.
  - Each core returns [128, 8] per-partition partial sums; host reduces
    and assembles the scalar loss.
"""

import sys

sys.path.insert(0, "/opt/trn_rl_repo")

import numpy as np

from concourse import bacc, bass, mybir, tile
from concourse.bass_utils import run_bass_kernel_spmd

N, D, R = 10000, 256, 4096
NCORES = 8
RPC = R // NCORES          # score rows per core = 512
SLC = N // NCORES          # mse rows per core = 1250
P = 128
NT = RPC // P              # score row-tiles per core = 4
MSE_FD = SLC * D // P      # 2500
F32 = mybir.dt.float32
U32 = mybir.dt.uint32

IDX_BITS = 14
IDX_MASK = (1 << IDX_BITS) - 1          # 0x3FFF
VAL_MASK = 0xFFFFFFFF ^ IDX_MASK        # 0xFFFFC000

_compiled = None


CN = 2                     # score chunks per row-tile
CF = N // CN               # chunk free dim = 5000


def _build_program():
    nc = bacc.Bacc("TRN2", target_bir_lowering=False, debug=False)

    scores = nc.dram_tensor("scores", [RPC, N], F32, kind="ExternalInput").ap()
    hsel = nc.dram_tensor("hsel", [RPC, D], F32, kind="ExternalInput").ap()
    hfull = nc.dram_tensor("hfull", [N, D], F32, kind="ExternalInput").ap()
    xs = nc.dram_tensor("xs", [P, MSE_FD], F32, kind="ExternalInput").ap()
    hs = nc.dram_tensor("hs", [P, MSE_FD], F32, kind="ExternalInput").ap()
    cs = nc.dram_tensor("cs", [P, MSE_FD], F32, kind="ExternalInput").ap()
    ms = nc.dram_tensor("ms", [P, MSE_FD], F32, kind="ExternalInput").ap()
    out = nc.dram_tensor("out", [P, 8], F32, kind="ExternalOutput").ap()

    with tile.TileContext(nc) as tc:
        with (
            tc.tile_pool(name="sc", bufs=4) as sc_pool,
            tc.tile_pool(name="small", bufs=NT) as small,
            tc.tile_pool(name="hp", bufs=NT) as hpool,
            tc.tile_pool(name="acc", bufs=1) as acc,
            tc.tile_pool(name="mse", bufs=1) as msep,
        ):
            res_t = acc.tile([P, 8], F32, tag="res")
            nc.vector.memset(res_t[:], 0.0)
            sim_cols = acc.tile([P, NT], F32, tag="simc")

            # DMA queue order (single HWDGE ring, FIFO): hsel + xs/hs first
            # (cheap, unblock early work), then the 8 score chunks (the
            # critical DVE supply), then cs/ms whose tail is short.
            hst = hpool.tile([P, NT * D], F32, tag="hst")
            nc.sync.dma_start(
                out=hst[:].rearrange("p (t d) -> p t d", t=NT),
                in_=hsel.rearrange("(t p) d -> p t d", p=P),
            )
            xt = msep.tile([P, MSE_FD], F32, tag="xt")
            ht = msep.tile([P, MSE_FD], F32, tag="ht")
            ct = msep.tile([P, MSE_FD], F32, tag="ct")
            mt = msep.tile([P, MSE_FD], F32, tag="mt")
            nc.sync.dma_start(out=xt[:], in_=xs)
            nc.sync.dma_start(out=ht[:], in_=hs)

            # phase A: per row-tile — chunked MAX8, merge, unpack, gather,
            # diff, square. All sim reductions deferred to phase B so the
            # DVE stream is never blocked by the gather chain.
            m8s, i3s, e3s, sq3s = [], [], [], []
            for t in range(NT):
                m8h = small.tile([P, CN * 8], F32, tag="m8h")
                for h in range(CN):
                    sc = sc_pool.tile([P, CF], F32, tag="sc")
                    nc.sync.dma_start(
                        out=sc[:],
                        in_=scores[t * P:(t + 1) * P, h * CF:(h + 1) * CF],
                    )
                    # packed = round14(-score) | col_idx; MAX8 ranks by
                    # value — one pass gives values AND (global) indices
                    nc.vector.max(out=m8h[:, h * 8:(h + 1) * 8], in_=sc[:])
                m8 = small.tile([P, 8], F32, tag="m8")
                nc.vector.max(out=m8[:], in_=m8h[:])
                i3 = small.tile([P, 3], U32, tag="i3")
                nc.vector.tensor_scalar(
                    out=i3[:], in0=m8[:, 0:3].bitcast(U32), scalar1=IDX_MASK,
                    scalar2=None, op0=mybir.AluOpType.bitwise_and,
                )
                v3 = small.tile([P, 3], F32, tag="v3")
                nc.vector.tensor_scalar(
                    out=v3[:].bitcast(U32), in0=m8[:, 0:3].bitcast(U32),
                    scalar1=VAL_MASK, scalar2=None,
                    op0=mybir.AluOpType.bitwise_and,
                )
                # gather the 3 neighbor H rows per partition row
                hn = hpool.tile([P, 3 * D], F32, tag="hn")
                for k in range(3):
                    nc.gpsimd.indirect_dma_start(
                        out=hn[:, k * D:(k + 1) * D],
                        out_offset=None,
                        in_=hfull,
                        in_offset=bass.IndirectOffsetOnAxis(ap=i3[:, k:k + 1], axis=0),
                    )
                dif = hpool.tile([P, 3 * D], F32, tag="dif")
                hb = hst[:, t * D:(t + 1) * D].unsqueeze(1).to_broadcast([P, 3, D])
                nc.gpsimd.tensor_tensor(
                    out=dif[:].rearrange("p (k d) -> p k d", k=3),
                    in0=hb, in1=hn[:].rearrange("p (k d) -> p k d", k=3),
                    op=mybir.AluOpType.subtract,
                )
                sq3 = hpool.tile([P, 3 * D], F32, tag="sq3")
                nc.scalar.activation(
                    out=sq3[:], in_=dif[:],
                    func=mybir.ActivationFunctionType.Square,
                )
                m8s.append(m8); i3s.append(i3); e3s.append(v3); sq3s.append(sq3)

                # mse residual chain on DVE, slotted into MAX8 slack /
                # after the last MAX8 (resid = (x - h + c) * m, in place)
                if t == 1:
                    nc.vector.tensor_tensor(out=xt[:], in0=xt[:], in1=ht[:],
                                            op=mybir.AluOpType.subtract)
                    nc.sync.dma_start(out=ct[:], in_=cs)
                if t == 2:
                    nc.vector.tensor_tensor(out=xt[:], in0=xt[:], in1=ct[:],
                                            op=mybir.AluOpType.add)
                    nc.sync.dma_start(out=mt[:], in_=ms)
                if t == 3:
                    nc.vector.tensor_tensor(out=xt[:], in0=xt[:], in1=mt[:],
                                            op=mybir.AluOpType.mult)

            # phase B: batched sim reductions (ACT ops grouped per function
            # to avoid LUT table reloads)
            e3v, s1s, r1s, nrms = [], [], [], []
            for t in range(NT):
                e3 = small.tile([P, 3], F32, tag="e3")
                # softmax over the 3 largest negated scores; values in
                # [~2, ~5.5] so exp() is safe in fp32 without a shift
                nc.scalar.activation(
                    out=e3[:], in_=e3s[t][:],
                    func=mybir.ActivationFunctionType.Exp,
                )
                e3v.append(e3)
            for t in range(NT):
                nrm2 = small.tile([P, 3], F32, tag="n2")
                nc.vector.tensor_reduce(
                    out=nrm2[:], in_=sq3s[t][:].rearrange("p (k d) -> p k d", k=3),
                    axis=mybir.AxisListType.X, op=mybir.AluOpType.add,
                )
                nrms.append(nrm2)
            for t in range(NT):
                nrm = small.tile([P, 3], F32, tag="nr")
                nc.scalar.sqrt(out=nrm[:], in_=nrms[t][:])
                nrms[t] = nrm
            for t in range(NT):
                s1 = small.tile([P, 1], F32, tag="s1")
                nc.vector.tensor_reduce(
                    out=s1[:], in_=e3v[t][:], axis=mybir.AxisListType.X,
                    op=mybir.AluOpType.add,
                )
                s1s.append(s1)
            for t in range(NT):
                r1 = small.tile([P, 1], F32, tag="r1")
                nc.vector.reciprocal(out=r1[:], in_=s1s[t][:])
                r1s.append(r1)
            for t in range(NT):
                # sim_t = (sum_k e3_k * nrm_k) / (sum_k e3_k)
                en = small.tile([P, 3], F32, tag="en")
                nc.vector.tensor_tensor(
                    out=en[:], in0=e3v[t][:], in1=nrms[t][:],
                    op=mybir.AluOpType.mult,
                )
                dot = small.tile([P, 1], F32, tag="dot")
                nc.vector.tensor_reduce(
                    out=dot[:], in_=en[:], axis=mybir.AxisListType.X,
                    op=mybir.AluOpType.add,
                )
                nc.vector.tensor_tensor(
                    out=sim_cols[:, t:t + 1], in0=dot[:], in1=r1s[t][:],
                    op=mybir.AluOpType.mult,
                )
            nc.vector.tensor_reduce(
                out=res_t[:, 0:1], in_=sim_cols[:], axis=mybir.AxisListType.X,
                op=mybir.AluOpType.add,
            )

            # squared-norm partials (ACT Square with free-dim accumulate)
            sq = msep.tile([P, MSE_FD], F32, tag="sq")
            nc.scalar.activation(out=sq[:], in_=ht[:],
                                 func=mybir.ActivationFunctionType.Square,
                                 accum_out=res_t[:, 2:3])
            nc.scalar.activation(out=sq[:], in_=ct[:],
                                 func=mybir.ActivationFunctionType.Square,
                                 accum_out=res_t[:, 3:4])
            nc.scalar.activation(out=sq[:], in_=xt[:],
                                 func=mybir.ActivationFunctionType.Square,
                                 accum_out=res_t[:, 1:2])

            nc.sync.dma_start(out=out, in_=res_t[:])

    nc.compile()
    return nc


def _get_program():
    global _compiled
    if _compiled is None:
        _compiled = _build_program()
    return _compiled


def _pack_scores(row_scores, mc):
    """Negate+gather score rows, round value to 9 mantissa bits and pack
    the column index into the low 14 bits."""
    neg = -row_scores[mc]                                   # [R, N] f32
    u = neg.view(np.uint32)
    packed = ((u + (1 << (IDX_BITS - 1))) & np.uint32(VAL_MASK)) | np.arange(
        N, dtype=np.uint32
    )[None, :]
    return packed.view(np.float32)


def _make_in_maps(X, H, C, M, row_scores, mc_rows):
    mc = np.asarray(mc_rows).astype(np.int64)
    scores_p = _pack_scores(np.ascontiguousarray(row_scores), mc)
    hsel_g = np.ascontiguousarray(H[mc])                    # [R, D]
    in_maps = []
    for c in range(NCORES):
        sl = slice(c * RPC, (c + 1) * RPC)
        rs = slice(c * SLC, (c + 1) * SLC)
        in_maps.append({
            "scores": scores_p[sl],
            "hsel": hsel_g[sl],
            "hfull": np.ascontiguousarray(H),
            "xs": np.ascontiguousarray(X[rs]).reshape(P, MSE_FD),
            "hs": np.ascontiguousarray(H[rs]).reshape(P, MSE_FD),
            "cs": np.ascontiguousarray(C[rs]).reshape(P, MSE_FD),
            "ms": np.ascontiguousarray(M[rs]).reshape(P, MSE_FD),
        })
    return in_maps


def _finish(results):
    parts = np.stack([r["out"] for r in results]).astype(np.float64)  # [8,128,8]
    tot = parts.sum(axis=(0, 1))
    loss = tot[1] + tot[0] + 0.1 * np.sqrt(tot[3]) + 0.01 * np.sqrt(tot[2])
    return np.array(loss, dtype=np.float32)


def kernel(X, H, C, M, T, nM, row_scores, mc_rows, **_unused):
    X = np.asarray(X, dtype=np.float32)
    H = np.asarray(H, dtype=np.float32)
    C = np.asarray(C, dtype=np.float32)
    M = np.asarray(M, dtype=np.float32)
    row_scores = np.asarray(row_scores, dtype=np.float32)
    nc = _get_program()
    in_maps = _make_in_maps(X, H, C, M, row_scores, mc_rows)
    res = run_bass_kernel_spmd(nc, in_maps, list(range(NCORES)))
    return _finish(res.results)


def run_traced(X, H, C, M, T, nM, row_scores, mc_rows, **_unused):
    """Like kernel() but returns (loss, BassKernelResults) with trace."""
    nc = _get_program()
    in_maps = _make_in_maps(
        np.asarray(X, dtype=np.float32), np.asarray(H, dtype=np.float32),
        np.asarray(C, dtype=np.float32), np.asarray(M, dtype=np.float32),
        np.asarray(row_scores, dtype=np.float32), mc_rows)
    try:
        res = run_bass_kernel_spmd(nc, in_maps, list(range(NCORES)), trace=True)
    except ModuleNotFoundError:
        res = run_bass_kernel_spmd(nc, in_maps, list(range(NCORES)))
    return _finish(res.results), res


# revision 7
# speedup vs baseline: 1.0603x; 1.0603x over previous
"""Trainium2 Bass kernel for nn_ClusterLoss (topk_masking).

Strategy (8 NeuronCores, data-parallel over the 4096 selected rows):
  - Host shards mc_rows and the corresponding gathered row_scores rows
    across cores (512 rows/core). The gathered rows are negated and the
    column index is packed into the low 14 mantissa bits (value rounded
    to the remaining 9 mantissa bits), so a single VectorE MAX8 pass
    yields both the 3 smallest scores and their column indices.
  - Device, per core: MAX8 per 128-row tile -> top-3 packed values;
    tiny bitwise unpack (indices + quantized values), softmax weights
    via ScalarE Exp, H[idx] gathered with indirect DMA, norm math
    spread across GpSimd/ScalarE/VectorE. Masked-MSE residual and
    squared-norm partials for a 1250-row slice of X/H/C/M.
  - Each core returns [128, 8] per-partition partial sums; host reduces
    and assembles the scalar loss.
"""

import sys

sys.path.insert(0, "/opt/trn_rl_repo")

import numpy as np

from concourse import bacc, bass, mybir, tile
from concourse.bass_utils import run_bass_kernel_spmd
from concourse.tile_rust import add_dep_helper

N, D, R = 10000, 256, 4096
NCORES = 8
RPC = R // NCORES          # score rows per core = 512
SLC = N // NCORES          # mse rows per core = 1250
P = 128
NT = RPC // P              # score row-tiles per core = 4
MSE_FD = SLC * D // P      # 2500
F32 = mybir.dt.float32
U32 = mybir.dt.uint32

IDX_BITS = 14
IDX_MASK = (1 << IDX_BITS) - 1          # 0x3FFF
VAL_MASK = 0xFFFFFFFF ^ IDX_MASK        # 0xFFFFC000

_compiled = None


CN = 2                     # score chunks per row-tile
CF = N // CN               # chunk free dim = 5000


def _build_program():
    nc = bacc.Bacc("TRN2", target_bir_lowering=False, debug=False)

    scores = nc.dram_tensor("scores", [RPC, N], F32, kind="ExternalInput").ap()
    hsel = nc.dram_tensor("hsel", [RPC, D], F32, kind="ExternalInput").ap()
    hfull = nc.dram_tensor("hfull", [N, D], F32, kind="ExternalInput").ap()
    xs = nc.dram_tensor("xs", [P, MSE_FD], F32, kind="ExternalInput").ap()
    hs = nc.dram_tensor("hs", [P, MSE_FD], F32, kind="ExternalInput").ap()
    cs = nc.dram_tensor("cs", [P, MSE_FD], F32, kind="ExternalInput").ap()
    ms = nc.dram_tensor("ms", [P, MSE_FD], F32, kind="ExternalInput").ap()
    out = nc.dram_tensor("out", [P, 8], F32, kind="ExternalOutput").ap()

    with tile.TileContext(nc) as tc:
        with (
            tc.tile_pool(name="sc", bufs=4) as sc_pool,
            tc.tile_pool(name="small", bufs=NT) as small,
            tc.tile_pool(name="hp", bufs=NT) as hpool,
            tc.tile_pool(name="acc", bufs=1) as acc,
            tc.tile_pool(name="mse", bufs=1) as msep,
        ):
            res_t = acc.tile([P, 8], F32, tag="res")
            nc.vector.memset(res_t[:], 0.0)
            sim_cols = acc.tile([P, NT], F32, tag="simc")

            # DMA queue order (single HWDGE ring, FIFO): hsel + xs/hs first
            # (cheap, unblock early work), then the 8 score chunks (the
            # critical DVE supply), then cs/ms whose tail is short.
            hst = hpool.tile([P, NT * D], F32, tag="hst")
            nc.sync.dma_start(
                out=hst[:].rearrange("p (t d) -> p t d", t=NT),
                in_=hsel.rearrange("(t p) d -> p t d", p=P),
            )
            xt = msep.tile([P, MSE_FD], F32, tag="xt")
            ht = msep.tile([P, MSE_FD], F32, tag="ht")
            ct = msep.tile([P, MSE_FD], F32, tag="ct")
            mt = msep.tile([P, MSE_FD], F32, tag="mt")
            nc.sync.dma_start(out=xt[:], in_=xs)
            nc.sync.dma_start(out=ht[:], in_=hs)

            # phase A: per row-tile — chunked MAX8, merge, unpack, gather,
            # diff, fused square+accum. All sim reductions deferred to
            # phase B so the DVE stream is never blocked by the gather
            # chain.
            i3s, v3s, nrm2s = [], [], []
            last_merge = None
            for t in range(NT):
                m8h = small.tile([P, CN * 8], F32, tag="m8h")
                for h in range(CN):
                    sc = sc_pool.tile([P, CF], F32, tag="sc")
                    nc.sync.dma_start(
                        out=sc[:],
                        in_=scores[t * P:(t + 1) * P, h * CF:(h + 1) * CF],
                    )
                    # packed = round14(-score) | col_idx; MAX8 ranks by
                    # value — one pass gives values AND (global) indices
                    nc.vector.max(out=m8h[:, h * 8:(h + 1) * 8], in_=sc[:])
                m8 = small.tile([P, 8], F32, tag="m8")
                last_merge = nc.vector.max(out=m8[:], in_=m8h[:])
                i3 = small.tile([P, 3], U32, tag="i3")
                nc.vector.tensor_scalar(
                    out=i3[:], in0=m8[:, 0:3].bitcast(U32), scalar1=IDX_MASK,
                    scalar2=None, op0=mybir.AluOpType.bitwise_and,
                )
                v3 = small.tile([P, 3], F32, tag="v3")
                nc.vector.tensor_scalar(
                    out=v3[:].bitcast(U32), in0=m8[:, 0:3].bitcast(U32),
                    scalar1=VAL_MASK, scalar2=None,
                    op0=mybir.AluOpType.bitwise_and,
                )
                # gather the 3 neighbor H rows (bf16-cast during SWDGE DMA
                # to halve the slow small-descriptor gather transfer)
                hn = hpool.tile([P, 3 * D], mybir.dt.bfloat16, tag="hn")
                for k in range(3):
                    nc.gpsimd.indirect_dma_start(
                        out=hn[:, k * D:(k + 1) * D],
                        out_offset=None,
                        in_=hfull,
                        in_offset=bass.IndirectOffsetOnAxis(ap=i3[:, k:k + 1], axis=0),
                    )
                dif = hpool.tile([P, 3 * D], F32, tag="dif")
                hb = hst[:, t * D:(t + 1) * D].unsqueeze(1).to_broadcast([P, 3, D])
                nc.gpsimd.tensor_tensor(
                    out=dif[:].rearrange("p (k d) -> p k d", k=3),
                    in0=hb, in1=hn[:].rearrange("p (k d) -> p k d", k=3),
                    op=mybir.AluOpType.subtract,
                )
                # ||diff||^2 per neighbor, fused on ACT (Square + free-dim
                # accumulate), squaring dif in place
                nrm2 = small.tile([P, 3], F32, tag="n2")
                for k in range(3):
                    nc.scalar.activation(
                        out=dif[:, k * D:(k + 1) * D],
                        in_=dif[:, k * D:(k + 1) * D],
                        func=mybir.ActivationFunctionType.Square,
                        accum_out=nrm2[:, k:k + 1],
                    )
                i3s.append(i3); v3s.append(v3); nrm2s.append(nrm2)

            nc.sync.dma_start(out=ct[:], in_=cs)
            nc.sync.dma_start(out=mt[:], in_=ms)
            # mse residual chain (resid = (x - h + c) * m, in place); TT1
            # can fill MAX8 slack, TT2/TT3 wait on cs/ms which land last
            nc.vector.tensor_tensor(out=xt[:], in0=xt[:], in1=ht[:],
                                    op=mybir.AluOpType.subtract)
            nc.vector.tensor_tensor(out=xt[:], in0=xt[:], in1=ct[:],
                                    op=mybir.AluOpType.add)
            nc.vector.tensor_tensor(out=xt[:], in0=xt[:], in1=mt[:],
                                    op=mybir.AluOpType.mult)

            # phase B: batched sim tail; every DVE op here is ordered after
            # the last MAX8 so a chain-blocked reduce can never stall the
            # score pipeline.
            def after_maxes(inst):
                add_dep_helper(inst.ins, last_merge.ins, sync=False,
                               reason="phase B after score maxes")

            e3v, s1s, r1s, nrms = [], [], [], []
            for t in range(NT):
                e3 = small.tile([P, 3], F32, tag="e3")
                # softmax over the 3 largest negated scores; values in
                # [~2, ~5.5] so exp() is safe in fp32 without a shift
                nc.scalar.activation(
                    out=e3[:], in_=v3s[t][:],
                    func=mybir.ActivationFunctionType.Exp,
                )
                e3v.append(e3)
            for t in range(NT):
                nrm = small.tile([P, 3], F32, tag="nr")
                nc.scalar.sqrt(out=nrm[:], in_=nrm2s[t][:])
                nrms.append(nrm)
            for t in range(NT):
                s1 = small.tile([P, 1], F32, tag="s1")
                after_maxes(nc.vector.tensor_reduce(
                    out=s1[:], in_=e3v[t][:], axis=mybir.AxisListType.X,
                    op=mybir.AluOpType.add,
                ))
                s1s.append(s1)
            for t in range(NT):
                r1 = small.tile([P, 1], F32, tag="r1")
                after_maxes(nc.vector.reciprocal(out=r1[:], in_=s1s[t][:]))
                r1s.append(r1)
            for t in range(NT):
                # sim_t = (sum_k e3_k * nrm_k) / (sum_k e3_k)
                en = small.tile([P, 3], F32, tag="en")
                after_maxes(nc.vector.tensor_tensor(
                    out=en[:], in0=e3v[t][:], in1=nrms[t][:],
                    op=mybir.AluOpType.mult,
                ))
                dot = small.tile([P, 1], F32, tag="dot")
                after_maxes(nc.vector.tensor_reduce(
                    out=dot[:], in_=en[:], axis=mybir.AxisListType.X,
                    op=mybir.AluOpType.add,
                ))
                after_maxes(nc.vector.tensor_tensor(
                    out=sim_cols[:, t:t + 1], in0=dot[:], in1=r1s[t][:],
                    op=mybir.AluOpType.mult,
                ))
            after_maxes(nc.vector.tensor_reduce(
                out=res_t[:, 0:1], in_=sim_cols[:], axis=mybir.AxisListType.X,
                op=mybir.AluOpType.add,
            ))

            # squared-norm partials (ACT Square with free-dim accumulate)
            sq = msep.tile([P, MSE_FD], F32, tag="sq")
            nc.scalar.activation(out=sq[:], in_=ht[:],
                                 func=mybir.ActivationFunctionType.Square,
                                 accum_out=res_t[:, 2:3])
            nc.scalar.activation(out=sq[:], in_=ct[:],
                                 func=mybir.ActivationFunctionType.Square,
                                 accum_out=res_t[:, 3:4])
            nc.scalar.activation(out=sq[:], in_=xt[:],
                                 func=mybir.ActivationFunctionType.Square,
                                 accum_out=res_t[:, 1:2])

            nc.sync.dma_start(out=out, in_=res_t[:])

    nc.compile()
    return nc


def _get_program():
    global _compiled
    if _compiled is None:
        _compiled = _build_program()
    return _compiled


def _pack_scores(row_scores, mc):
    """Negate+gather score rows, round value to 9 mantissa bits and pack
    the column index into the low 14 bits."""
    neg = -row_scores[mc]                                   # [R, N] f32
    u = neg.view(np.uint32)
    packed = ((u + (1 << (IDX_BITS - 1))) & np.uint32(VAL_MASK)) | np.arange(
        N, dtype=np.uint32
    )[None, :]
    return packed.view(np.float32)


def _make_in_maps(X, H, C, M, row_scores, mc_rows):
    mc = np.asarray(mc_rows).astype(np.int64)
    scores_p = _pack_scores(np.ascontiguousarray(row_scores), mc)
    hsel_g = np.ascontiguousarray(H[mc])                    # [R, D]
    in_maps = []
    for c in range(NCORES):
        sl = slice(c * RPC, (c + 1) * RPC)
        rs = slice(c * SLC, (c + 1) * SLC)
        in_maps.append({
            "scores": scores_p[sl],
            "hsel": hsel_g[sl],
            "hfull": np.ascontiguousarray(H),
            "xs": np.ascontiguousarray(X[rs]).reshape(P, MSE_FD),
            "hs": np.ascontiguousarray(H[rs]).reshape(P, MSE_FD),
            "cs": np.ascontiguousarray(C[rs]).reshape(P, MSE_FD),
            "ms": np.ascontiguousarray(M[rs]).reshape(P, MSE_FD),
        })
    return in_maps


def _finish(results):
    parts = np.stack([r["out"] for r in results]).astype(np.float64)  # [8,128,8]
    tot = parts.sum(axis=(0, 1))
    loss = tot[1] + tot[0] + 0.1 * np.sqrt(tot[3]) + 0.01 * np.sqrt(tot[2])
    return np.array(loss, dtype=np.float32)


def kernel(X, H, C, M, T, nM, row_scores, mc_rows, **_unused):
    X = np.asarray(X, dtype=np.float32)
    H = np.asarray(H, dtype=np.float32)
    C = np.asarray(C, dtype=np.float32)
    M = np.asarray(M, dtype=np.float32)
    row_scores = np.asarray(row_scores, dtype=np.float32)
    nc = _get_program()
    in_maps = _make_in_maps(X, H, C, M, row_scores, mc_rows)
    res = run_bass_kernel_spmd(nc, in_maps, list(range(NCORES)))
    return _finish(res.results)


def run_traced(X, H, C, M, T, nM, row_scores, mc_rows, **_unused):
    """Like kernel() but returns (loss, BassKernelResults) with trace."""
    nc = _get_program()
    in_maps = _make_in_maps(
        np.asarray(X, dtype=np.float32), np.asarray(H, dtype=np.float32),
        np.asarray(C, dtype=np.float32), np.asarray(M, dtype=np.float32),
        np.asarray(row_scores, dtype=np.float32), mc_rows)
    try:
        res = run_bass_kernel_spmd(nc, in_maps, list(range(NCORES)), trace=True)
    except ModuleNotFoundError:
        res = run_bass_kernel_spmd(nc, in_maps, list(range(NCORES)))
    return _finish(res.results), res


# revision 8
# speedup vs baseline: 1.2737x; 1.2013x over previous
"""Trainium2 Bass kernel for nn_ClusterLoss (topk_masking).

Strategy (8 NeuronCores, data-parallel over the 4096 selected rows):
  - Host shards mc_rows and the corresponding gathered row_scores rows
    across cores (512 rows/core). The gathered rows are negated and the
    column index is packed into the low 14 mantissa bits (value rounded
    to the remaining 9 mantissa bits), so a single VectorE MAX8 pass
    yields both the 3 smallest scores and their column indices.
  - Device, per core: MAX8 per 128-row tile -> top-3 packed values;
    tiny bitwise unpack (indices + quantized values), softmax weights
    via ScalarE Exp, H[idx] gathered with indirect DMA, norm math
    spread across GpSimd/ScalarE/VectorE. Masked-MSE residual and
    squared-norm partials for a 1250-row slice of X/H/C/M.
  - Each core returns [128, 8] per-partition partial sums; host reduces
    and assembles the scalar loss.
"""

import sys

sys.path.insert(0, "/opt/trn_rl_repo")

import numpy as np

from concourse import bacc, bass, mybir, tile
from concourse.bass_utils import run_bass_kernel_spmd
from concourse.tile_rust import add_dep_helper

N, D, R = 10000, 256, 4096
NCORES = 8
RPC = R // NCORES          # score rows per core = 512
SLC = N // NCORES          # mse rows per core = 1250
P = 128
NT = RPC // P              # score row-tiles per core = 4
MSE_FD = SLC * D // P      # 2500
F32 = mybir.dt.float32
U32 = mybir.dt.uint32

IDX_BITS = 14
IDX_MASK = (1 << IDX_BITS) - 1          # 0x3FFF
VAL_MASK = 0xFFFFFFFF ^ IDX_MASK        # 0xFFFFC000

_compiled = None


CN = 4                     # score chunks per row-tile
CF = N // CN               # chunk free dim = 2500


def _build_program():
    nc = bacc.Bacc("TRN2", target_bir_lowering=False, debug=False)

    scores = nc.dram_tensor("scores", [RPC, N], F32, kind="ExternalInput").ap()
    hsel = nc.dram_tensor("hsel", [P, NT * D], F32, kind="ExternalInput").ap()
    hfull = nc.dram_tensor("hfull", [N, D], F32, kind="ExternalInput").ap()
    xs = nc.dram_tensor("xs", [P, MSE_FD], F32, kind="ExternalInput").ap()
    hs = nc.dram_tensor("hs", [P, MSE_FD], F32, kind="ExternalInput").ap()
    cs = nc.dram_tensor("cs", [P, MSE_FD], F32, kind="ExternalInput").ap()
    ms = nc.dram_tensor("ms", [P, MSE_FD], F32, kind="ExternalInput").ap()
    out = nc.dram_tensor("out", [P, 8], F32, kind="ExternalOutput").ap()

    with tile.TileContext(nc) as tc:
        with (
            tc.tile_pool(name="sc", bufs=6) as sc_pool,
            tc.tile_pool(name="small", bufs=NT) as small,
            tc.tile_pool(name="hp", bufs=NT) as hpool,
            tc.tile_pool(name="acc", bufs=1) as acc,
            tc.tile_pool(name="mse", bufs=1) as msep,
        ):
            res_t = acc.tile([P, 8], F32, tag="res")
            nc.vector.memset(res_t[:], 0.0)
            sim_cols = acc.tile([P, NT], F32, tag="simc")

            # DMA queue order (single HWDGE ring, FIFO): hsel + xs/hs first
            # (cheap, unblock early work), then the 8 score chunks (the
            # critical DVE supply), then cs/ms whose tail is short.
            xt = msep.tile([P, MSE_FD], F32, tag="xt")
            ht = msep.tile([P, MSE_FD], F32, tag="ht")
            ct = msep.tile([P, MSE_FD], F32, tag="ct")
            mt = msep.tile([P, MSE_FD], F32, tag="mt")
            nc.sync.dma_start(out=xt[:], in_=xs)
            nc.sync.dma_start(out=ht[:], in_=hs)
            # hsel is host-packed to [P, NT*D] (partition p holds rows
            # p, p+128, ... ) so this lands as one fast contiguous DMA
            hst = hpool.tile([P, NT * D], F32, tag="hst")
            nc.sync.dma_start(out=hst[:], in_=hsel)

            # phase A: per row-tile — chunked MAX8, merge, unpack, gather,
            # diff, fused square+accum. All sim reductions deferred to
            # phase B so the DVE stream is never blocked by the gather
            # chain.
            i3s, v3s, nrm2s = [], [], []
            last_merge = None
            for t in range(NT):
                m8h = small.tile([P, CN * 8], F32, tag="m8h")
                for h in range(CN):
                    sc = sc_pool.tile([P, CF], F32, tag="sc")
                    nc.sync.dma_start(
                        out=sc[:],
                        in_=scores[t * P:(t + 1) * P, h * CF:(h + 1) * CF],
                    )
                    # packed = round14(-score) | col_idx; MAX8 ranks by
                    # value — one pass gives values AND (global) indices
                    nc.vector.max(out=m8h[:, h * 8:(h + 1) * 8], in_=sc[:])
                m8 = small.tile([P, 8], F32, tag="m8")
                last_merge = nc.vector.max(out=m8[:], in_=m8h[:])
                i3 = small.tile([P, 3], U32, tag="i3")
                nc.vector.tensor_scalar(
                    out=i3[:], in0=m8[:, 0:3].bitcast(U32), scalar1=IDX_MASK,
                    scalar2=None, op0=mybir.AluOpType.bitwise_and,
                )
                v3 = small.tile([P, 3], F32, tag="v3")
                last_bits = nc.vector.tensor_scalar(
                    out=v3[:].bitcast(U32), in0=m8[:, 0:3].bitcast(U32),
                    scalar1=VAL_MASK, scalar2=None,
                    op0=mybir.AluOpType.bitwise_and,
                )
                # gather the 3 neighbor H rows per partition row
                hn = hpool.tile([P, 3 * D], F32, tag="hn")
                for k in range(3):
                    nc.gpsimd.indirect_dma_start(
                        out=hn[:, k * D:(k + 1) * D],
                        out_offset=None,
                        in_=hfull,
                        in_offset=bass.IndirectOffsetOnAxis(ap=i3[:, k:k + 1], axis=0),
                    )
                dif = hpool.tile([P, 3 * D], F32, tag="dif")
                hb = hst[:, t * D:(t + 1) * D].unsqueeze(1).to_broadcast([P, 3, D])
                nc.gpsimd.tensor_tensor(
                    out=dif[:].rearrange("p (k d) -> p k d", k=3),
                    in0=hb, in1=hn[:].rearrange("p (k d) -> p k d", k=3),
                    op=mybir.AluOpType.subtract,
                )
                # ||diff||^2 per neighbor, fused on ACT (Square + free-dim
                # accumulate), squaring dif in place
                nrm2 = small.tile([P, 3], F32, tag="n2")
                for k in range(3):
                    nc.scalar.activation(
                        out=dif[:, k * D:(k + 1) * D],
                        in_=dif[:, k * D:(k + 1) * D],
                        func=mybir.ActivationFunctionType.Square,
                        accum_out=nrm2[:, k:k + 1],
                    )
                i3s.append(i3); v3s.append(v3); nrm2s.append(nrm2)

            nc.sync.dma_start(out=ct[:], in_=cs)
            nc.sync.dma_start(out=mt[:], in_=ms)
            # mse residual chain (resid = (x - h + c) * m, in place); TT1
            # can fill MAX8 slack, TT2/TT3 wait on cs/ms which land last
            nc.vector.tensor_tensor(out=xt[:], in0=xt[:], in1=ht[:],
                                    op=mybir.AluOpType.subtract)
            tt2 = nc.vector.tensor_tensor(out=xt[:], in0=xt[:], in1=ct[:],
                                          op=mybir.AluOpType.add)
            tt3 = nc.vector.tensor_tensor(out=xt[:], in0=xt[:], in1=mt[:],
                                          op=mybir.AluOpType.mult)
            # keep the last tile's unpack (and so its gather kickoff) ahead
            # of the mse chain on the DVE stream
            add_dep_helper(tt2.ins, last_bits.ins, sync=False,
                           reason="mse TTs after last unpack")

            # phase B: batched sim tail; every DVE op here is ordered after
            # the last MAX8 so a chain-blocked reduce can never stall the
            # score pipeline.
            def after_maxes(inst):
                add_dep_helper(inst.ins, last_merge.ins, sync=False,
                               reason="phase B after score maxes")

            e3v, s1s, r1s, nrms = [], [], [], []
            for t in range(NT):
                e3 = small.tile([P, 3], F32, tag="e3")
                # softmax over the 3 largest negated scores; values in
                # [~2, ~5.5] so exp() is safe in fp32 without a shift
                nc.scalar.activation(
                    out=e3[:], in_=v3s[t][:],
                    func=mybir.ActivationFunctionType.Exp,
                )
                e3v.append(e3)
            for t in range(NT):
                nrm = small.tile([P, 3], F32, tag="nr")
                nc.scalar.sqrt(out=nrm[:], in_=nrm2s[t][:])
                nrms.append(nrm)
            for t in range(NT):
                s1 = small.tile([P, 1], F32, tag="s1")
                after_maxes(nc.vector.tensor_reduce(
                    out=s1[:], in_=e3v[t][:], axis=mybir.AxisListType.X,
                    op=mybir.AluOpType.add,
                ))
                s1s.append(s1)
            for t in range(NT):
                r1 = small.tile([P, 1], F32, tag="r1")
                after_maxes(nc.vector.reciprocal(out=r1[:], in_=s1s[t][:]))
                r1s.append(r1)
            for t in range(NT):
                # sim_t = (sum_k e3_k * nrm_k) / (sum_k e3_k)
                en = small.tile([P, 3], F32, tag="en")
                after_maxes(nc.vector.tensor_tensor(
                    out=en[:], in0=e3v[t][:], in1=nrms[t][:],
                    op=mybir.AluOpType.mult,
                ))
                dot = small.tile([P, 1], F32, tag="dot")
                after_maxes(nc.vector.tensor_reduce(
                    out=dot[:], in_=en[:], axis=mybir.AxisListType.X,
                    op=mybir.AluOpType.add,
                ))
                after_maxes(nc.vector.tensor_tensor(
                    out=sim_cols[:, t:t + 1], in0=dot[:], in1=r1s[t][:],
                    op=mybir.AluOpType.mult,
                ))
            after_maxes(nc.vector.tensor_reduce(
                out=res_t[:, 0:1], in_=sim_cols[:], axis=mybir.AxisListType.X,
                op=mybir.AluOpType.add,
            ))

            # squared-norm partials (ACT Square with free-dim accumulate)
            sq = msep.tile([P, MSE_FD], F32, tag="sq")
            nc.scalar.activation(out=sq[:], in_=ht[:],
                                 func=mybir.ActivationFunctionType.Square,
                                 accum_out=res_t[:, 2:3])
            nc.scalar.activation(out=sq[:], in_=ct[:],
                                 func=mybir.ActivationFunctionType.Square,
                                 accum_out=res_t[:, 3:4])
            nc.scalar.activation(out=sq[:], in_=xt[:],
                                 func=mybir.ActivationFunctionType.Square,
                                 accum_out=res_t[:, 1:2])

            nc.sync.dma_start(out=out, in_=res_t[:])

    nc.compile()
    return nc


def _get_program():
    global _compiled
    if _compiled is None:
        _compiled = _build_program()
    return _compiled


def _pack_scores(row_scores, mc):
    """Negate+gather score rows, round value to 9 mantissa bits and pack
    the column index into the low 14 bits."""
    neg = -row_scores[mc]                                   # [R, N] f32
    u = neg.view(np.uint32)
    packed = ((u + (1 << (IDX_BITS - 1))) & np.uint32(VAL_MASK)) | np.arange(
        N, dtype=np.uint32
    )[None, :]
    return packed.view(np.float32)


def _make_in_maps(X, H, C, M, row_scores, mc_rows):
    mc = np.asarray(mc_rows).astype(np.int64)
    scores_p = _pack_scores(np.ascontiguousarray(row_scores), mc)
    hsel_g = H[mc]                                          # [R, D]
    in_maps = []
    for c in range(NCORES):
        sl = slice(c * RPC, (c + 1) * RPC)
        rs = slice(c * SLC, (c + 1) * SLC)
        in_maps.append({
            "scores": scores_p[sl],
            "hsel": np.ascontiguousarray(
                hsel_g[sl].reshape(NT, P, D).transpose(1, 0, 2).reshape(
                    P, NT * D)),
            "hfull": np.ascontiguousarray(H),
            "xs": np.ascontiguousarray(X[rs]).reshape(P, MSE_FD),
            "hs": np.ascontiguousarray(H[rs]).reshape(P, MSE_FD),
            "cs": np.ascontiguousarray(C[rs]).reshape(P, MSE_FD),
            "ms": np.ascontiguousarray(M[rs]).reshape(P, MSE_FD),
        })
    return in_maps


def _finish(results):
    parts = np.stack([r["out"] for r in results]).astype(np.float64)  # [8,128,8]
    tot = parts.sum(axis=(0, 1))
    loss = tot[1] + tot[0] + 0.1 * np.sqrt(tot[3]) + 0.01 * np.sqrt(tot[2])
    return np.array(loss, dtype=np.float32)


def kernel(X, H, C, M, T, nM, row_scores, mc_rows, **_unused):
    X = np.asarray(X, dtype=np.float32)
    H = np.asarray(H, dtype=np.float32)
    C = np.asarray(C, dtype=np.float32)
    M = np.asarray(M, dtype=np.float32)
    row_scores = np.asarray(row_scores, dtype=np.float32)
    nc = _get_program()
    in_maps = _make_in_maps(X, H, C, M, row_scores, mc_rows)
    res = run_bass_kernel_spmd(nc, in_maps, list(range(NCORES)))
    return _finish(res.results)


def run_traced(X, H, C, M, T, nM, row_scores, mc_rows, **_unused):
    """Like kernel() but returns (loss, BassKernelResults) with trace."""
    nc = _get_program()
    in_maps = _make_in_maps(
        np.asarray(X, dtype=np.float32), np.asarray(H, dtype=np.float32),
        np.asarray(C, dtype=np.float32), np.asarray(M, dtype=np.float32),
        np.asarray(row_scores, dtype=np.float32), mc_rows)
    try:
        res = run_bass_kernel_spmd(nc, in_maps, list(range(NCORES)), trace=True)
    except ModuleNotFoundError:
        res = run_bass_kernel_spmd(nc, in_maps, list(range(NCORES)))
    return _finish(res.results), res


# revision 9
# speedup vs baseline: 1.3243x; 1.0397x over previous
"""Trainium2 Bass kernel for nn_ClusterLoss (topk_masking).

Strategy (8 NeuronCores, data-parallel over the 4096 selected rows):
  - Host shards mc_rows and the corresponding gathered row_scores rows
    across cores (512 rows/core). The gathered rows are negated and the
    column index is packed into the low 14 mantissa bits (value rounded
    to the remaining 9 mantissa bits), so a single VectorE MAX8 pass
    yields both the 3 smallest scores and their column indices.
  - Device, per core: MAX8 per 128-row tile -> top-3 packed values;
    tiny bitwise unpack (indices + quantized values), softmax weights
    via ScalarE Exp, H[idx] gathered with indirect DMA, norm math
    spread across GpSimd/ScalarE/VectorE. Masked-MSE residual and
    squared-norm partials for a 1250-row slice of X/H/C/M.
  - Each core returns [128, 8] per-partition partial sums; host reduces
    and assembles the scalar loss.
"""

import sys

sys.path.insert(0, "/opt/trn_rl_repo")

import numpy as np

from concourse import bacc, bass, mybir, tile
from concourse.bass_utils import run_bass_kernel_spmd
from concourse.tile_rust import add_dep_helper

N, D, R = 10000, 256, 4096
NCORES = 8
RPC = R // NCORES          # score rows per core = 512
SLC = N // NCORES          # mse rows per core = 1250
P = 128
NT = RPC // P              # score row-tiles per core = 4
MSE_FD = SLC * D // P      # 2500
F32 = mybir.dt.float32
U32 = mybir.dt.uint32

IDX_BITS = 14
IDX_MASK = (1 << IDX_BITS) - 1          # 0x3FFF
VAL_MASK = 0xFFFFFFFF ^ IDX_MASK        # 0xFFFFC000

_compiled = None


CN = 4                     # score chunks per row-tile
CF = N // CN               # chunk free dim = 2500


def _build_program():
    nc = bacc.Bacc("TRN2", target_bir_lowering=False, debug=False)

    scores = nc.dram_tensor("scores", [RPC, N], F32, kind="ExternalInput").ap()
    hsel = nc.dram_tensor("hsel", [P, NT * D], F32, kind="ExternalInput").ap()
    hfull = nc.dram_tensor("hfull", [N, D], F32, kind="ExternalInput").ap()
    xs = nc.dram_tensor("xs", [P, MSE_FD], F32, kind="ExternalInput").ap()
    hs = nc.dram_tensor("hs", [P, MSE_FD], F32, kind="ExternalInput").ap()
    cs = nc.dram_tensor("cs", [P, MSE_FD], F32, kind="ExternalInput").ap()
    ms = nc.dram_tensor("ms", [P, MSE_FD], F32, kind="ExternalInput").ap()
    out = nc.dram_tensor("out", [P, 8], F32, kind="ExternalOutput").ap()

    with tile.TileContext(nc) as tc:
        with (
            tc.tile_pool(name="sc", bufs=6) as sc_pool,
            tc.tile_pool(name="small", bufs=NT) as small,
            tc.tile_pool(name="hp", bufs=NT) as hpool,
            tc.tile_pool(name="acc", bufs=1) as acc,
            tc.tile_pool(name="mse", bufs=1) as msep,
        ):
            res_t = acc.tile([P, 8], F32, tag="res")
            nc.vector.memset(res_t[:], 0.0)
            sim_cols = acc.tile([P, NT], F32, tag="simc")

            # DMA queue order (single HWDGE ring, FIFO): hsel + xs/hs first
            # (cheap, unblock early work), then the 8 score chunks (the
            # critical DVE supply), then cs/ms whose tail is short.
            xt = msep.tile([P, MSE_FD], F32, tag="xt")
            ht = msep.tile([P, MSE_FD], F32, tag="ht")
            ct = msep.tile([P, MSE_FD], F32, tag="ct")
            mt = msep.tile([P, MSE_FD], F32, tag="mt")
            nc.sync.dma_start(out=xt[:], in_=xs)
            nc.sync.dma_start(out=ht[:], in_=hs)
            # hsel is host-packed to [P, NT*D] (partition p holds rows
            # p, p+128, ... ) so this lands as one fast contiguous DMA
            hst = hpool.tile([P, NT * D], F32, tag="hst")
            nc.sync.dma_start(out=hst[:], in_=hsel)

            # phase A: per row-tile — chunked MAX8, merge, unpack, gather,
            # diff, fused square+accum. All sim reductions deferred to
            # phase B so the DVE stream is never blocked by the gather
            # chain. The last tile's chunks taper so its final MAX8 (on
            # the critical tail) is short.
            v3all = acc.tile([P, NT * 3], F32, tag="v3all")
            nrm2all = acc.tile([P, NT * 3], F32, tag="n2all")
            i3s = []
            last_merge = None
            last_bits = None
            nrm2_t3 = None
            for t in range(NT):
                chunks = [2500] * 4 if t < NT - 1 else [2500, 2500, 2500, 1875, 625]
                m8h = small.tile([P, len(chunks) * 8], F32, tag="m8h")
                col = 0
                for h, w in enumerate(chunks):
                    sc = sc_pool.tile([P, w], F32, tag="sc")
                    nc.sync.dma_start(
                        out=sc[:],
                        in_=scores[t * P:(t + 1) * P, col:col + w],
                    )
                    col += w
                    # packed = round14(-score) | col_idx; MAX8 ranks by
                    # value — one pass gives values AND (global) indices
                    nc.vector.max(out=m8h[:, h * 8:(h + 1) * 8], in_=sc[:])
                m8 = small.tile([P, 8], F32, tag="m8")
                last_merge = nc.vector.max(out=m8[:], in_=m8h[:])
                i3 = small.tile([P, 3], U32, tag="i3")
                nc.vector.tensor_scalar(
                    out=i3[:], in0=m8[:, 0:3].bitcast(U32), scalar1=IDX_MASK,
                    scalar2=None, op0=mybir.AluOpType.bitwise_and,
                )
                last_bits = nc.vector.tensor_scalar(
                    out=v3all[:, t * 3:(t + 1) * 3].bitcast(U32),
                    in0=m8[:, 0:3].bitcast(U32),
                    scalar1=VAL_MASK, scalar2=None,
                    op0=mybir.AluOpType.bitwise_and,
                )
                # gather the 3 neighbor H rows per partition row
                hn = hpool.tile([P, 3 * D], F32, tag="hn")
                for k in range(3):
                    nc.gpsimd.indirect_dma_start(
                        out=hn[:, k * D:(k + 1) * D],
                        out_offset=None,
                        in_=hfull,
                        in_offset=bass.IndirectOffsetOnAxis(ap=i3[:, k:k + 1], axis=0),
                    )
                dif = hpool.tile([P, 3 * D], F32, tag="dif")
                hb = hst[:, t * D:(t + 1) * D].unsqueeze(1).to_broadcast([P, 3, D])
                nc.gpsimd.tensor_tensor(
                    out=dif[:].rearrange("p (k d) -> p k d", k=3),
                    in0=hb, in1=hn[:].rearrange("p (k d) -> p k d", k=3),
                    op=mybir.AluOpType.subtract,
                )
                # ||diff||^2 per neighbor. t0-2: fused on ACT (Square +
                # free-dim accumulate). t3 (critical tail): on DVE to keep
                # the ACT Square-table reload off the critical path.
                if t < NT - 1:
                    for k in range(3):
                        nc.scalar.activation(
                            out=dif[:, k * D:(k + 1) * D],
                            in_=dif[:, k * D:(k + 1) * D],
                            func=mybir.ActivationFunctionType.Square,
                            accum_out=nrm2all[:, t * 3 + k:t * 3 + k + 1],
                        )
                else:
                    sqd = hpool.tile([P, 3 * D], F32, tag="sqd")
                    nc.vector.tensor_tensor(
                        out=sqd[:], in0=dif[:], in1=dif[:],
                        op=mybir.AluOpType.mult,
                    )
                    nrm2_t3 = nc.vector.tensor_reduce(
                        out=nrm2all[:, t * 3:(t + 1) * 3],
                        in_=sqd[:].rearrange("p (k d) -> p k d", k=3),
                        axis=mybir.AxisListType.X, op=mybir.AluOpType.add,
                    )
                i3s.append(i3)

            nc.sync.dma_start(out=ct[:], in_=cs)
            nc.sync.dma_start(out=mt[:], in_=ms)
            # mse residual chain (resid = (x - h + c) * m, in place); TT1
            # can fill MAX8 slack, TT2/TT3 wait on cs/ms which land last
            nc.vector.tensor_tensor(out=xt[:], in0=xt[:], in1=ht[:],
                                    op=mybir.AluOpType.subtract)
            tt2 = nc.vector.tensor_tensor(out=xt[:], in0=xt[:], in1=ct[:],
                                          op=mybir.AluOpType.add)
            tt3 = nc.vector.tensor_tensor(out=xt[:], in0=xt[:], in1=mt[:],
                                          op=mybir.AluOpType.mult)
            # keep the last tile's unpack (and so its gather kickoff) ahead
            # of the mse chain on the DVE stream
            add_dep_helper(tt2.ins, last_bits.ins, sync=False,
                           reason="mse TTs after last unpack")

            # phase B: consolidated sim tail — one wide op per step (one
            # Exp and one Sqrt table load total), all DVE ops ordered
            # after the last MAX8 merge.
            def after_maxes(inst):
                add_dep_helper(inst.ins, last_merge.ins, sync=False,
                               reason="phase B after score maxes")

            e3all = acc.tile([P, NT * 3], F32, tag="e3all")
            # softmax over the 3 largest negated scores; values in
            # [~2, ~5.5] so exp() is safe in fp32 without a shift
            nc.scalar.activation(
                out=e3all[:], in_=v3all[:],
                func=mybir.ActivationFunctionType.Exp,
            )
            nrmall = acc.tile([P, NT * 3], F32, tag="nrmall")
            nc.scalar.sqrt(out=nrmall[:], in_=nrm2all[:])
            s1 = acc.tile([P, NT], F32, tag="s1")
            after_maxes(nc.vector.tensor_reduce(
                out=s1[:], in_=e3all[:].rearrange("p (t k) -> p t k", k=3),
                axis=mybir.AxisListType.X, op=mybir.AluOpType.add,
            ))
            r1 = acc.tile([P, NT], F32, tag="r1")
            after_maxes(nc.vector.reciprocal(out=r1[:], in_=s1[:]))
            en = acc.tile([P, NT * 3], F32, tag="en")
            after_maxes(nc.vector.tensor_tensor(
                out=en[:], in0=e3all[:], in1=nrmall[:],
                op=mybir.AluOpType.mult,
            ))
            dot = acc.tile([P, NT], F32, tag="dot")
            after_maxes(nc.vector.tensor_reduce(
                out=dot[:], in_=en[:].rearrange("p (t k) -> p t k", k=3),
                axis=mybir.AxisListType.X, op=mybir.AluOpType.add,
            ))
            after_maxes(nc.vector.tensor_tensor(
                out=sim_cols[:], in0=dot[:], in1=r1[:],
                op=mybir.AluOpType.mult,
            ))
            after_maxes(nc.vector.tensor_reduce(
                out=res_t[:, 0:1], in_=sim_cols[:], axis=mybir.AxisListType.X,
                op=mybir.AluOpType.add,
            ))

            # squared-norm partials (ACT Square with free-dim accumulate)
            sq = msep.tile([P, MSE_FD], F32, tag="sq")
            nc.scalar.activation(out=sq[:], in_=ht[:],
                                 func=mybir.ActivationFunctionType.Square,
                                 accum_out=res_t[:, 2:3])
            nc.scalar.activation(out=sq[:], in_=ct[:],
                                 func=mybir.ActivationFunctionType.Square,
                                 accum_out=res_t[:, 3:4])
            nc.scalar.activation(out=sq[:], in_=xt[:],
                                 func=mybir.ActivationFunctionType.Square,
                                 accum_out=res_t[:, 1:2])

            nc.sync.dma_start(out=out, in_=res_t[:])

    nc.compile()
    return nc


def _get_program():
    global _compiled
    if _compiled is None:
        _compiled = _build_program()
    return _compiled


def _pack_scores(row_scores, mc):
    """Negate+gather score rows, round value to 9 mantissa bits and pack
    the column index into the low 14 bits."""
    neg = -row_scores[mc]                                   # [R, N] f32
    u = neg.view(np.uint32)
    packed = ((u + (1 << (IDX_BITS - 1))) & np.uint32(VAL_MASK)) | np.arange(
        N, dtype=np.uint32
    )[None, :]
    return packed.view(np.float32)


def _make_in_maps(X, H, C, M, row_scores, mc_rows):
    mc = np.asarray(mc_rows).astype(np.int64)
    scores_p = _pack_scores(np.ascontiguousarray(row_scores), mc)
    hsel_g = H[mc]                                          # [R, D]
    in_maps = []
    for c in range(NCORES):
        sl = slice(c * RPC, (c + 1) * RPC)
        rs = slice(c * SLC, (c + 1) * SLC)
        in_maps.append({
            "scores": scores_p[sl],
            "hsel": np.ascontiguousarray(
                hsel_g[sl].reshape(NT, P, D).transpose(1, 0, 2).reshape(
                    P, NT * D)),
            "hfull": np.ascontiguousarray(H),
            "xs": np.ascontiguousarray(X[rs]).reshape(P, MSE_FD),
            "hs": np.ascontiguousarray(H[rs]).reshape(P, MSE_FD),
            "cs": np.ascontiguousarray(C[rs]).reshape(P, MSE_FD),
            "ms": np.ascontiguousarray(M[rs]).reshape(P, MSE_FD),
        })
    return in_maps


def _finish(results):
    parts = np.stack([r["out"] for r in results]).astype(np.float64)  # [8,128,8]
    tot = parts.sum(axis=(0, 1))
    loss = tot[1] + tot[0] + 0.1 * np.sqrt(tot[3]) + 0.01 * np.sqrt(tot[2])
    return np.array(loss, dtype=np.float32)


def kernel(X, H, C, M, T, nM, row_scores, mc_rows, **_unused):
    X = np.asarray(X, dtype=np.float32)
    H = np.asarray(H, dtype=np.float32)
    C = np.asarray(C, dtype=np.float32)
    M = np.asarray(M, dtype=np.float32)
    row_scores = np.asarray(row_scores, dtype=np.float32)
    nc = _get_program()
    in_maps = _make_in_maps(X, H, C, M, row_scores, mc_rows)
    res = run_bass_kernel_spmd(nc, in_maps, list(range(NCORES)))
    return _finish(res.results)


def run_traced(X, H, C, M, T, nM, row_scores, mc_rows, **_unused):
    """Like kernel() but returns (loss, BassKernelResults) with trace."""
    nc = _get_program()
    in_maps = _make_in_maps(
        np.asarray(X, dtype=np.float32), np.asarray(H, dtype=np.float32),
        np.asarray(C, dtype=np.float32), np.asarray(M, dtype=np.float32),
        np.asarray(row_scores, dtype=np.float32), mc_rows)
    try:
        res = run_bass_kernel_spmd(nc, in_maps, list(range(NCORES)), trace=True)
    except ModuleNotFoundError:
        res = run_bass_kernel_spmd(nc, in_maps, list(range(NCORES)))
    return _finish(res.results), res
